# revision 30
# baseline (speedup 1.0000x reference)
"""Multi-head attention (B=2, S=2048, D=1024, H=16) on 8 TRN2 NeuronCores.

Sharding: core c handles batch b = c//4 and heads [4*(c%4), 4*(c%4)+4) —
tensor-parallel over heads x data-parallel over batch.  Each core computes a
partial output projection (its heads' contribution); the host sums the 4
partials per batch and adds b_out.

On-device layout (per core; bf16 matmul operands, fp32 PSUM/softmax math):
  - qk projection computed transposed: qkT [512, S] with row chunks
    [q_h0|q_h1, k_h0|k_h1, q_h2|q_h3, k_h2|k_h3] so the two heads of a pair
    occupy partitions 0-63 / 64-127 and their K=64 score matmuls run
    concurrently in distinct PE row-groups.
  - scores computed transposed: expT[sk, sq] = exp(scale * kT.T @ qT); the
    softmax denominator comes free from a ones-column appended to v in the
    attn@v matmul (out row 64 = sum over sk of expT).  No max-subtraction:
    scores*scale is ~N(0,1) so fp32 exp cannot overflow.
  - attn@v: valuesT_unnorm [65, sq] = v_aug.T @ expT accumulated over sk
    chunks in reversed order (one ACT wait, then back-to-back so LDWEIGHTS
    hides); normalized by the denominator reciprocal, broadcast across
    partitions via a DRAM-bounce stride-0 DMA.  The v-bias is linear through
    the output projection and is added on the host instead.
  - output projection out_partial = valuesT.T @ WoutT is emitted per sq
    block as soon as its vT slices land, overlapping the attention phase.
  - scheduling: each block's attn@v is deferred one block (its exps are then
    provably done -> waitless PE chains) and each normalize tail two blocks
    (the ~4us DVE reciprocal never stalls the PE).
"""
import sys

sys.path.insert(0, "/opt/trn_rl_repo")

import numpy as np

B, S, D = 2, 2048, 1024
H, Hd = 16, 64
P = 128
NKC = D // P      # 8 contraction chunks for the projections
NSC = S // P      # 16 sequence chunks of 128
SQB = 512         # sq block size
NSQB = S // SQB   # 4

_CACHE = {}


def _build_nc():
    import concourse.mybir as mybir
    import concourse.tile as tile
    from concourse import bacc

    f32 = mybir.dt.float32
    f32r = mybir.dt.float32r
    bf16 = mybir.dt.bfloat16
    AF = mybir.ActivationFunctionType

    nc = bacc.Bacc(None, target_bir_lowering=False, debug=False)

    yT_d = nc.dram_tensor("yT", [D, S], bf16, kind="ExternalInput")[:]
    Wqk_d = nc.dram_tensor("WqkT", [D, 512], bf16, kind="ExternalInput")[:]
    bqk_d = nc.dram_tensor("bqk", [P, 4], f32, kind="ExternalInput")[:]
    Wv_d = nc.dram_tensor("WvT", [D, 256], bf16, kind="ExternalInput")[:]
    Wout_d = nc.dram_tensor("WoutT", [256, D], bf16, kind="ExternalInput")[:]
    out_d = nc.dram_tensor("out", [S, D], f32, kind="ExternalOutput")[:]

    with tile.TileContext(nc) as tc:
        with (
            tc.tile_pool(name="const", bufs=1) as const,
            tc.tile_pool(name="persist", bufs=1) as persist,
        ):
            Wout_sb = const.tile([P, 2, D], bf16)
            nc.sync.dma_start(Wout_sb[:], Wout_d.rearrange("(kc p) e -> p kc e", p=P))
            bqk_sb = const.tile([P, 4], f32)
            nc.sync.dma_start(bqk_sb[:], bqk_d)

            qkT_sb = persist.tile([P, 4, S], bf16)
            v_sb = persist.tile([P, NSC, 4, 65], bf16)
            vT_sb = persist.tile([P, 2, S], bf16)
            ones_f32 = const.tile([P, 1], f32)
            nc.any.memset(ones_f32[:], 1.0)
            nc.vector.tensor_copy(
                v_sb[:, :, :, 64:65],
                ones_f32.unsqueeze(1).unsqueeze(1).to_broadcast(
                    (P, NSC, 4, 1)))

            # ---- phase 1: qk projection (v-proj overlaps phase 2's first
            # ACT-paced score block) ----
            p1 = ctx_p1 = tc.alloc_tile_pool(name="p1", bufs=1)
            with tc.tile_pool(name="p1ps", bufs=4, space="PSUM") as p1ps:
                Wqk_sb = p1.tile([P, NKC, 512], bf16)
                nc.sync.dma_start(
                    Wqk_sb[:], Wqk_d.rearrange("(kc p) e -> p kc e", p=P))
                Wv_sb = p1.tile([P, NKC, 256], bf16)
                nc.sync.dma_start(
                    Wv_sb[:], Wv_d.rearrange("(kc p) e -> p kc e", p=P))
                yT_sb = p1.tile([P, NKC, S], bf16)
                yTr = yT_d.rearrange("(kc p) s -> p kc s", p=P)
                for kc in range(NKC):
                    nc.sync.dma_start(yT_sb[:, kc, :], yTr[:, kc, :])

                for m in range(4):
                    for sb in range(4):
                        ps = p1ps.tile([P, 512], f32, tag="proj")
                        for kc in range(NKC):
                            nc.tensor.matmul(
                                ps[:],
                                Wqk_sb[:, kc, m * P:(m + 1) * P],
                                yT_sb[:, kc, sb * 512:(sb + 1) * 512],
                                start=(kc == 0), stop=(kc == NKC - 1))
                        nc.scalar.activation(
                            qkT_sb[:, m, sb * 512:(sb + 1) * 512], ps[:],
                            AF.Identity, bias=bqk_sb[:, m:m + 1])

            # ---- phase 2: attention (per head pair, per sq block) ----
            # After each head's attn@v, the unnormalized values and the
            # reciprocal of the denominator row are staged to SBUF right away
            # (freeing the PSUM slot); the normalize tail (broadcast matmul +
            # multiply + bias + DMA into vT_sb) is deferred by two blocks so
            # the PE never waits on the ~4us DVE reciprocal.
            with (
                tc.tile_pool(name="p2e", bufs=4) as p2e,
                tc.tile_pool(name="p2s", bufs=2) as p2s,
                tc.tile_pool(name="p2ps", bufs=4, space="PSUM") as p2ps,
                tc.tile_pool(name="p2dram", bufs=4, space="DRAM") as p2dram,
                tc.tile_pool(name="p2psv", bufs=2, space="PSUM") as p2psv,
            ):
                pending = []

                def v_proj():
                    for sc in range(NSC):
                        psv = p2psv.tile([P, 256], f32, tag="vproj", bufs=2,
                                         name="psv")
                        for kc in range(NKC):
                            nc.tensor.matmul(
                                psv[:],
                                yT_sb[:, kc, sc * P:(sc + 1) * P],
                                Wv_sb[:, kc, :],
                                start=(kc == 0), stop=(kc == NKC - 1))
                        nc.vector.tensor_copy(
                            v_sb[:, sc, :, 0:64],
                            psv.rearrange("p (i d) -> p i d", i=4))

                def normalize_tail(p, sqb, sub, vals, rdram):
                    sq = slice(sqb * SQB, (sqb + 1) * SQB)
                    # broadcast recip row across 64 partitions by re-reading
                    # the DRAM copy with a stride-0 partition dimension
                    # (v-bias is folded into the output on the host)
                    rbs = p2s.tile([64, SQB], f32, tag="rbs", name="rbs")
                    nc.sync.dma_start(rbs[:], rdram.to_broadcast((64, SQB)))
                    vtmp = p2s.tile([64, SQB], bf16, tag="vtmp", name="vtmp")
                    nc.vector.tensor_mul(vtmp[:], vals[:], rbs[:])
                    nc.sync.dma_start(
                        vT_sb[sub * 64:(sub + 1) * 64, p, sq], vtmp[:])

                def out_proj(sqb):
                    # output projection for the s-chunks of one sq block —
                    # emitted as soon as both pairs' vT slices are written
                    for sc in range(sqb * 4, sqb * 4 + 4):
                        for nb in range(2):
                            pso = p2psv.tile([P, 512], f32, tag="vproj",
                                             name="pso")
                            for kc in range(2):
                                nc.tensor.matmul(
                                    pso[:],
                                    vT_sb[:, kc, sc * P:(sc + 1) * P],
                                    Wout_sb[:, kc,
                                            nb * 512:(nb + 1) * 512],
                                    start=(kc == 0), stop=(kc == 1))
                            ost = p2s.tile([P, 512], f32, tag="ost",
                                           name="ost", bufs=3)
                            nc.scalar.copy(ost[:], pso[:])
                            nc.sync.dma_start(
                                out_d[sc * P:(sc + 1) * P,
                                      nb * 512:(nb + 1) * 512], ost[:])

                def attn_v(p, sqb, ex):
                    """attn@v for a completed score/exp block, plus immediate
                    staging of values+reciprocal to SBUF."""
                    for sub in range(2):
                        i = 2 * p + sub
                        psv2 = p2psv.tile([P, SQB], f32, tag="vt",
                                          name="psv2")
                        # reversed order: only the first matmul waits on ACT
                        # (all exps of this tile done); the rest issue
                        # back-to-back so the PE pulls LDWEIGHTS ahead and
                        # the array stays busy
                        for mk in range(NSC - 1, -1, -1):
                            nc.tensor.matmul(
                                psv2[0:65, :],
                                v_sb[:, mk, i, :],
                                ex[sub][:, mk, :],
                                start=(mk == NSC - 1), stop=(mk == 0))
                        vals = p2s.tile([64, SQB], f32, tag="vals",
                                        name="vals", bufs=6)
                        nc.vector.tensor_copy(vals[:], psv2[0:64, :])
                        rb = p2s.tile([P, SQB], f32, tag="rb", name="rb")
                        nc.vector.reciprocal(rb[64:65, :], psv2[64:65, :])
                        rdram = p2dram.tile([1, SQB], f32, name="rdram")
                        nc.sync.dma_start(rdram[:], rb[64:65, :])
                        pending.append((p, sqb, sub, vals, rdram))

                prev = None
                for p in range(2):
                    for sqb in range(NSQB):
                        sq = slice(sqb * SQB, (sqb + 1) * SQB)
                        exa = p2e.tile([P, NSC, SQB], bf16, tag="exp")
                        exb = p2e.tile([P, NSC, SQB], bf16, tag="exp")
                        ex = (exa, exb)
                        # two sk-chunks share one 2-bank PSUM tile; a single
                        # exp activation covers both (halves ACT op count).
                        # h0/h64 matmuls adjacent -> subarray concurrency.
                        for mj in range(NSC // 2):
                            pss = [
                                p2ps.tile([P, 2, SQB], f32, tag="score",
                                          bufs=2, name="pss")
                                for _ in range(2)]
                            for half in range(2):
                                mk = 2 * mj + half
                                for sub in range(2):
                                    prt = slice(sub * 64, (sub + 1) * 64)
                                    nc.tensor.matmul(
                                        pss[sub][:, half, :],
                                        qkT_sb[prt, 2 * p + 1,
                                               mk * P:(mk + 1) * P],
                                        qkT_sb[prt, 2 * p, sq])
                            for sub in range(2):
                                nc.scalar.activation(
                                    ex[sub][:, 2 * mj:2 * mj + 2, :],
                                    pss[sub][:], AF.Exp, scale=0.125)
                        # previous block's attn@v interleaves with this
                        # block's ACT-paced scores on the PE; the v
                        # projection fills the first block's gaps
                        if prev is not None:
                            attn_v(*prev)
                        else:
                            v_proj()
                        prev = (p, sqb, ex)
                        while len(pending) > 2:
                            pp, psqb, psub, pvals, prd = pending.pop(0)
                            normalize_tail(pp, psqb, psub, pvals, prd)
                            if pp == 1 and psub == 1:
                                out_proj(psqb)
                attn_v(*prev)
                for args in pending:
                    normalize_tail(*args)
                    if args[0] == 1 and args[2] == 1:
                        out_proj(args[1])

            ctx_p1.release()

    nc.compile()
    return nc


def _get_nc():
    if "nc" not in _CACHE:
        _CACHE["nc"] = _build_nc()
    return _CACHE["nc"]


def _host_prep(y, W_qkv, b_qkv, W_out, c):
    b = c // 4
    q = c % 4
    hs = [4 * q + i for i in range(4)]

    def Wrow(h, part):
        return W_qkv[h * 192 + part * 64: h * 192 + (part + 1) * 64]

    def brow(h, part):
        return b_qkv[h * 192 + part * 64: h * 192 + (part + 1) * 64]

    qk_rows = np.concatenate([
        Wrow(hs[0], 0), Wrow(hs[1], 0), Wrow(hs[0], 1), Wrow(hs[1], 1),
        Wrow(hs[2], 0), Wrow(hs[3], 0), Wrow(hs[2], 1), Wrow(hs[3], 1)],
        axis=0)
    bqk_flat = np.concatenate([
        brow(hs[0], 0), brow(hs[1], 0), brow(hs[0], 1), brow(hs[1], 1),
        brow(hs[2], 0), brow(hs[3], 0), brow(hs[2], 1), brow(hs[3], 1)],
        axis=0)
    import ml_dtypes

    bf = ml_dtypes.bfloat16
    WqkT = np.ascontiguousarray(qk_rows.T.astype(bf))        # [1024, 512]
    bqk = np.ascontiguousarray(bqk_flat.reshape(4, P).T)     # [128, 4]
    WvT = np.ascontiguousarray(
        np.concatenate([Wrow(h, 2) for h in hs], axis=0).T.astype(bf))
    dsl = np.concatenate([np.arange(h * 64, (h + 1) * 64) for h in hs])
    WoutT = np.ascontiguousarray(W_out[:, dsl].T.astype(bf))  # [256, 1024]
    yT = np.ascontiguousarray(y[b].T.astype(bf))             # [1024, 2048]
    return {"yT": yT, "WqkT": WqkT, "bqk": bqk, "WvT": WvT,
            "WoutT": WoutT}


def _gather(results, b_qkv, W_out, b_out):
    parts = [results[c]["out"] for c in range(8)]
    # v-bias commutes through the output projection: fold it host-side
    bv_full = b_qkv.reshape(16, 3, 64)[:, 2, :].reshape(1024)
    bias = b_out + bv_full @ W_out.T
    return np.stack([
        parts[0] + parts[1] + parts[2] + parts[3] + bias,
        parts[4] + parts[5] + parts[6] + parts[7] + bias,
    ]).astype(np.float32)


def kernel(y, W_qkv, b_qkv, W_out, b_out):
    from concourse.bass_utils import run_bass_kernel_spmd

    y = np.ascontiguousarray(np.asarray(y, dtype=np.float32))
    W_qkv = np.ascontiguousarray(np.asarray(W_qkv, dtype=np.float32))
    b_qkv = np.ascontiguousarray(np.asarray(b_qkv, dtype=np.float32))
    W_out = np.ascontiguousarray(np.asarray(W_out, dtype=np.float32))
    b_out = np.asarray(b_out, dtype=np.float32)

    nc = _get_nc()
    in_maps = [_host_prep(y, W_qkv, b_qkv, W_out, c) for c in range(8)]
    res = run_bass_kernel_spmd(nc, in_maps, core_ids=list(range(8)))
    return _gather(res.results, b_qkv, W_out, b_out)
